# revision 14
# baseline (speedup 1.0000x reference)
"""Distributed Trainium2 kernel for causal multi-head attention with RoPE.

Problem: B=2, S=2048, E=2048, N=16 heads, H=128 head-dim.
Sharding: 8 cores = 2 (batch, data axis) x 4 (head groups, model axis).
Each core:
  phase 1: Q/K/V projections for its 4 heads (bf16 matmuls, f32 accum),
           RoPE applied to q^T/k^T in [H, S] layout.
  phase 2: causal attention per head (exact softmax per 128-row block,
           no max subtraction -- logits are O(5) so exp is safe in f32).
           Row-blocks are processed in pairs so the pv stage uses N=256
           matmuls (half the LDWEIGHTS+MATMUL count).
  phase 3: two AllGathers inside each 4-core group (heads 0-1 after they
           finish, heads 2-3 at the end) collect a^T for all 16 heads;
           the first AG overlaps the second half of phase 2. Each core
           then computes the output projection for a different 512-wide
           E-slice (its WO shard) over all s, producing o[2048, 512] f32.
Host side re-assembles the 8 [2048, 512] E-chunks into [2, 2048, 2048].
"""

import contextlib

import numpy as np
import ml_dtypes

import concourse.bass as bass
import concourse.mybir as mybir
import concourse.tile as tile
from concourse import bacc
from concourse.bass_utils import run_bass_kernel_spmd
from concourse.masks import make_identity, make_causal_mask

B, S, E, N, H = 2, 2048, 2048, 16, 128
P = 128
NCORES = 8
NH_LOC = N // 4          # 4 heads per core
ECHUNK = E // 4          # 512 output-embedding columns per core
EC = E // P              # 16 contraction chunks
ST = S // P              # 16 seq tiles of 128
F32 = mybir.dt.float32
BF16 = mybir.dt.bfloat16

REPLICA_GROUPS = [[0, 1, 2, 3], [4, 5, 6, 7]]

TRACE = False
LAST_RESULTS = None
PT_VIA_DMA = False       # transpose P via DMA xbar instead of PE+copy
POOL_CFG = {"elt": 6, "lt": 2, "av": 3, "sm": 2, "bc": 1}


def _rope_tables():
    """cos^T / sin^T tables [H, S] f32, sign-folded and scaled by 128**-0.25.

    Matches reference.sine_table computed in f32 (angles formed with f32
    arithmetic, sin/cos evaluated in f64 of the f32 angle).
    """
    fraction = np.arange(0, H, 2, dtype=np.float32) / np.float32(H)
    timescale = np.float32(10000.0) ** fraction
    inv = (np.float32(1.0) / timescale).astype(np.float32)
    ang = (np.arange(S, dtype=np.float32)[:, None] * inv[None, :]).astype(np.float32)
    ang = np.concatenate([ang, ang], axis=1)        # [S, H]
    sin = np.sin(ang.astype(np.float64))
    cos = np.cos(ang.astype(np.float64))
    scale = float(H) ** -0.25
    cosT = (cos.T * scale).astype(np.float32)        # [H, S]
    sinT = (sin.T * scale).astype(np.float32)
    sinT[:H // 2] *= -1.0                            # sign of rotate_half folded in
    return np.ascontiguousarray(cosT), np.ascontiguousarray(sinT)


def _phase1(nc, tc, qT_sc, kT_sc, v_sc, cos_sb, sin_sb, cos_srcs,
            xqT_r, xkT_r, wq_r, wk_r, wv_r, sfx=""):
    """QKV proj + RoPE -> per-s-chunk qT/kT [h, n, 512] and v [t, tt, nh]."""
    with (
        tc.tile_pool(name="wqkv" + sfx, bufs=1) as w_pool,
        tc.tile_pool(name="xin" + sfx, bufs=2) as x_pool,
        tc.tile_pool(name="rope_tmp" + sfx, bufs=3) as tmp_pool,
        tc.tile_pool(name="ph1_psum" + sfx, bufs=4, space="PSUM") as pp1,
    ):
        # per-4-ec-chunk weight loads: the first q chain starts after a
        # 0.5MB chunk instead of the full 2MB tensor
        wq_sb = w_pool.tile([P, EC, NH_LOC * H], BF16)
        wk_sb = w_pool.tile([P, EC, NH_LOC * H], BF16)
        for j in range(4):
            cs = slice(4 * j, 4 * j + 4)
            nc.sync.dma_start(wq_sb[:, cs, :], wq_r[:, cs, :])
            nc.sync.dma_start(wk_sb[:, cs, :], wk_r[:, cs, :])
        wv_sb = w_pool.tile([P, EC, NH_LOC * H], BF16)

        def rope(ps, dst, cols):
            """dst = ps*cos + shift128(ps)*sin  (bf16 out), cols into S."""
            w = cols.stop - cols.start
            t_sin = tmp_pool.tile([P, 512], F32, tag="t_sin")
            nc.vector.tensor_tensor(
                out=t_sin[0:64, :w], in0=ps[64:128, :w],
                in1=sin_sb[0:64, cols], op=mybir.AluOpType.mult)
            nc.vector.tensor_tensor(
                out=t_sin[64:128, :w], in0=ps[0:64, :w],
                in1=sin_sb[64:128, cols], op=mybir.AluOpType.mult)
            t_cos = tmp_pool.tile([P, 512], F32, tag="t_cos")
            nc.vector.tensor_tensor(
                out=t_cos[:, :w], in0=ps[:, :w],
                in1=cos_sb[:, cols], op=mybir.AluOpType.mult)
            nc.vector.tensor_add(out=dst, in0=t_cos[:, :w], in1=t_sin[:, :w])

        for sc in range(4):                     # 512-col s-chunks
            cols = slice(sc * 512, (sc + 1) * 512)
            # 4 strip tiles of 4 e-chunks each -> matmuls start after the
            # first strip lands instead of the full 2MB transfer.
            xq_st, xk_st = [], []
            for j in range(4):
                xq = x_pool.tile([P, 4, 512], BF16, tag=f"xq{j}")
                nc.sync.dma_start(xq[:], xqT_r[:, 4 * j:4 * j + 4, cols])
                xq_st.append(xq)
                xk = x_pool.tile([P, 4, 512], BF16, tag=f"xk{j}")
                nc.sync.dma_start(xk[:], xkT_r[:, 4 * j:4 * j + 4, cols])
                xk_st.append(xk)
            if sc == 0:
                # deferred loads: not needed until RoPE / the v chains,
                # so they queue behind the first q/k strips
                nc.sync.dma_start(cos_sb[:], cos_srcs[0])
                nc.sync.dma_start(sin_sb[:], cos_srcs[1])
                nc.sync.dma_start(wv_sb[:], wv_r)

            for n in range(NH_LOC):
                hs = slice(n * H, (n + 1) * H)
                psq = pp1.tile([P, 512], F32, tag="ps")
                for ec in range(EC):
                    nc.tensor.matmul(
                        psq, wq_sb[:, ec, hs], xq_st[ec // 4][:, ec % 4, :],
                        start=(ec == 0), stop=(ec == EC - 1))
                rope(psq, qT_sc[sc][:, n, :], cols)
                psk = pp1.tile([P, 512], F32, tag="ps")
                for ec in range(EC):
                    nc.tensor.matmul(
                        psk, wk_sb[:, ec, hs], xk_st[ec // 4][:, ec % 4, :],
                        start=(ec == 0), stop=(ec == EC - 1))
                rope(psk, kT_sc[sc][:, n, :], cols)

            for tt in range(4):                 # v for 4 t-tiles
                tsl = slice(tt * P, (tt + 1) * P)
                psv = pp1.tile([P, 512], F32, tag="ps")
                for ec in range(EC):
                    nc.tensor.matmul(
                        psv, xk_st[ec // 4][:, ec % 4, tsl], wv_sb[:, ec, :],
                        start=(ec == 0), stop=(ec == EC - 1))
                nc.scalar.copy(out=v_sc[sc][:, tt, :], in_=psv[:])


def _attn_heads_v2(nc, tc, pools, heads, qT_sc, kT_sc, v_sc, consts, ag_in,
                   no_sm=False):
    """Causal attention, transposed-score formulation.

    Scores are computed directly as L^T[t, s] (lhsT = k^T block, rhs = q^T
    chunk, N=512), exp goes PSUM -> SBUF bf16 with no transpose/copy, and
    a^T[h, s] accumulates with wide pv matmuls (lhsT = v block).  Row-sums
    (softmax denominators) come from ones-column matmuls accumulated next
    to the pv chain; normalization multiplies a^T by an outer-product
    broadcast (K=1 matmul) of the reciprocals.
    """
    (elt_pool, rc_pool, bc_pool, at_pool,
     lt_psum, av_psum, sm_psum, bc_psum) = pools
    maskT, ones_col, ones_row = consts

    def flush(pend):
        """Deferred normalization: by now rc is long since computed on DVE,
        so the bc broadcast matmul never stalls the PE."""
        n, sc, avp, rc = pend
        bcp = bc_psum.tile([P, 512], F32, tag="bc")
        nc.tensor.matmul(bcp, ones_row[:], rc[:], start=True, stop=True)
        bcs = bc_pool.tile([P, 512], F32, tag="bcs")
        nc.vector.tensor_copy(out=bcs[:], in_=bcp[:])
        at = at_pool.tile([P, 512], BF16, tag="at")
        nc.vector.tensor_mul(out=at[:], in0=avp[:], in1=bcs[:])
        nc.sync.dma_start(
            ag_in.ap()[n * P:(n + 1) * P, sc * 512:(sc + 1) * 512], at[:])

    pending = None
    for sc in range(4):                         # 512-wide s-chunks
        for n in heads:
            hs = slice(n * H, (n + 1) * H)
            ntb = 4 * sc + 4                    # causal t-blocks for chunk
            avp = av_psum.tile([P, 512], F32, tag="av")
            smp = sm_psum.tile([1, 512], F32, tag="sm")
            for tb in range(ntb):
                w = 512 if tb < 4 * sc else 512 - (tb - 4 * sc) * P
                col0 = 512 - w
                kblk = kT_sc[tb // 4][:, n, (tb % 4) * P:(tb % 4 + 1) * P]
                lt = lt_psum.tile([P, 512], F32, tag="lt")
                nc.tensor.matmul(
                    lt[:, :w], kblk, qT_sc[sc][:, n, col0:512],
                    start=True, stop=True)
                elt = elt_pool.tile([P, 512], BF16, tag="elt")
                nc.scalar.activation(
                    out=elt[:, :w], in_=lt[:, :w],
                    func=mybir.ActivationFunctionType.Exp)
                if tb >= 4 * sc:                # diagonal: zero t > s
                    nc.vector.tensor_mul(
                        out=elt[:, 0:P], in0=elt[:, 0:P], in1=maskT[:])
                nc.tensor.matmul(
                    avp[:, col0:512], v_sc[tb // 4][:, tb % 4, hs],
                    elt[:, :w],
                    start=(tb == 0), stop=(tb == ntb - 1),
                    skip_group_check=True)
                if not no_sm:
                    nc.tensor.matmul(
                        smp[:, col0:512], ones_col[:], elt[:, :w],
                        start=(tb == 0), stop=(tb == ntb - 1),
                        skip_group_check=True)
            if no_sm:
                at = at_pool.tile([P, 512], BF16, tag="at")
                nc.vector.tensor_copy(out=at[:], in_=avp[:])
                nc.sync.dma_start(
                    ag_in.ap()[n * P:(n + 1) * P,
                               sc * 512:(sc + 1) * 512], at[:])
                continue
            rc = rc_pool.tile([1, 512], F32, tag="rc")
            nc.vector.reciprocal(rc[:], smp[:])
            if pending is not None:
                flush(pending)
            pending = (n, sc, avp, rc)
    if pending is not None:
        flush(pending)


def _attn_heads(nc, tc, pools, heads, qT_sc, kT_sc, v_sc, dmask, ident, ag_in):
    """Causal attention for the given local heads; writes a^T into ag_in."""
    (probs_pool, ssum_pool, ptsb_pool, atsb_pool,
     sc_psum, pt_psum, av_psum) = pools

    def qT(n, rows):
        return qT_sc[rows.start // 512][:, n,
                                        rows.start % 512:
                                        rows.start % 512 + (rows.stop - rows.start)]

    def softmax_block(n, i, probs):
        """logits + exp + causal 0/1-mask + normalize for row-block i.

        The diagonal block is masked multiplicatively on the cheap bf16
        side (exp of unmasked logits is finite, ~e^6 max), and its chunk
        row-sum is recomputed from the masked bf16 probs.
        """
        span = (i + 1) * P
        nch = (span + 511) // 512
        dj = (span - P) // 512                  # chunk holding the diagonal
        sums = ssum_pool.tile([P, 4], F32, tag="sums")
        rows = slice(i * P, (i + 1) * P)
        for j in range(nch):
            w = min(512, span - j * 512)
            sp = sc_psum.tile([P, 512], F32, tag="sp")
            nc.tensor.matmul(
                sp[:, :w], qT(n, rows), kT_sc[j][:, n, :w],
                start=True, stop=True)
            nc.scalar.activation(
                out=probs[:, j * 512:j * 512 + w],
                in_=sp[:, :w],
                func=mybir.ActivationFunctionType.Exp,
                accum_out=(None if j == dj else sums[:, j:j + 1]))
        off = (i % 4) * P
        dsl = slice(span - P, span)
        nc.vector.tensor_mul(
            out=probs[:, dsl], in0=probs[:, dsl], in1=dmask[:])
        wd = min(512, span - dj * 512)
        nc.vector.reduce_sum(
            out=sums[:, dj:dj + 1], in_=probs[:, dj * 512:dj * 512 + wd],
            axis=mybir.AxisListType.X)
        recip = ssum_pool.tile([P, 1], F32, tag="recip")
        if nch > 1:
            tot = ssum_pool.tile([P, 1], F32, tag="tot")
            nc.vector.reduce_sum(
                out=tot[:], in_=sums[:, :nch], axis=mybir.AxisListType.X)
            nc.vector.reciprocal(recip[:], tot[:])
        else:
            nc.vector.reciprocal(recip[:], sums[:, 0:1])
        for j in range(nch):
            w = min(512, span - j * 512)
            nc.vector.tensor_scalar_mul(
                probs[:, j * 512:j * 512 + w],
                probs[:, j * 512:j * 512 + w], recip[:])

    for m in range(ST // 2):                    # paired row-blocks (2m, 2m+1)
        for n in heads:
            hs = slice(n * H, (n + 1) * H)
            i0, i1 = 2 * m, 2 * m + 1
            probs0 = probs_pool.tile([P, S], BF16, tag="probs0")
            probs1 = probs_pool.tile([P, S], BF16, tag="probs1")
            softmax_block(n, i0, probs0)
            softmax_block(n, i1, probs1)

            # a^T = v.T @ P^T for both blocks: pv rhs is [t, 256]
            # (128 cols per block); i1's diagonal t-block rides in the
            # last transpose group (nblk0 is odd, so it has a free slot).
            avp = av_psum.tile([P, 2 * P], F32, tag="av")
            nblk0 = i0 + 1
            for g in range(0, nblk0, 2):        # 2 t-blocks per transpose grp
                gw = min(2, nblk0 - g)
                last = (g + 2 >= nblk0)
                pts = ptsb_pool.tile([P, 512], BF16, tag="pts")
                if PT_VIA_DMA:
                    for q in range(gw):
                        tb = g + q
                        nc.sync.dma_start(
                            out=pts[:, q * 256:q * 256 + P],
                            in_=probs0[:, tb * P:(tb + 1) * P], transpose=True)
                        nc.sync.dma_start(
                            out=pts[:, q * 256 + P:q * 256 + 2 * P],
                            in_=probs1[:, tb * P:(tb + 1) * P], transpose=True)
                    if last:
                        nc.sync.dma_start(
                            out=pts[:, gw * 256:gw * 256 + P],
                            in_=probs1[:, i1 * P:(i1 + 1) * P], transpose=True)
                else:
                    ptp = pt_psum.tile([P, 512], BF16, tag="pt")
                    for q in range(gw):
                        tb = g + q
                        nc.tensor.transpose(
                            ptp[:, q * 256:q * 256 + P],
                            probs0[:, tb * P:(tb + 1) * P], ident)
                        nc.tensor.transpose(
                            ptp[:, q * 256 + P:q * 256 + 2 * P],
                            probs1[:, tb * P:(tb + 1) * P], ident)
                    cw = gw * 256
                    if last:                    # diag t-block of i1
                        nc.tensor.transpose(
                            ptp[:, cw:cw + P],
                            probs1[:, i1 * P:(i1 + 1) * P], ident)
                        cw += P
                    nc.any.tensor_copy(out=pts[:, :cw], in_=ptp[:, :cw])
                for q in range(gw):
                    tb = g + q
                    nc.tensor.matmul(
                        avp, v_sc[tb // 4][:, tb % 4, hs],
                        pts[:, q * 256:(q + 1) * 256],
                        start=(tb == 0), stop=False, skip_group_check=True)
                if last:
                    nc.tensor.matmul(
                        avp[:, P:2 * P], v_sc[i1 // 4][:, i1 % 4, hs],
                        pts[:, gw * 256:gw * 256 + P],
                        start=False, stop=True, skip_group_check=True)

            at = atsb_pool.tile([P, 2 * P], BF16, tag="at")
            nc.any.tensor_copy(out=at[:], in_=avp[:])
            nc.sync.dma_start(
                ag_in.ap()[n * P:(n + 1) * P, i0 * P:(i0 + 2) * P], at[:])


def _build(reps=1, with_cc=True, loop_trips=0, parts=("p1", "p2", "p3"),
           no_sm=False):
    nc = bacc.Bacc("TRN2", target_bir_lowering=False, debug=False,
                   num_devices=NCORES)

    xqT = nc.dram_tensor("xqT", [E, S], BF16, kind="ExternalInput")
    xkT = nc.dram_tensor("xkT", [E, S], BF16, kind="ExternalInput")
    wq = nc.dram_tensor("wq", [E, NH_LOC * H], BF16, kind="ExternalInput")
    wk = nc.dram_tensor("wk", [E, NH_LOC * H], BF16, kind="ExternalInput")
    wv = nc.dram_tensor("wv", [E, NH_LOC * H], BF16, kind="ExternalInput")
    wo = nc.dram_tensor("wo", [N * H, ECHUNK], BF16, kind="ExternalInput")
    cosT = nc.dram_tensor("cosT", [H, S], F32, kind="ExternalInput")
    sinT = nc.dram_tensor("sinT", [H, S], F32, kind="ExternalInput")
    out = nc.dram_tensor("out", [S, ECHUNK], F32, kind="ExternalOutput")

    ag_in = nc.dram_tensor("ag_in", [NH_LOC * H, S], BF16)
    ag_out1 = nc.dram_tensor("ag_out1", [4, 3 * H, S], BF16)
    ag_out2 = nc.dram_tensor("ag_out2", [4, 1 * H, S], BF16)

    xqT_r = xqT.ap().rearrange("(eo p) s -> p eo s", p=P)    # [128, 16, 2048]
    xkT_r = xkT.ap().rearrange("(eo p) s -> p eo s", p=P)
    wq_r = wq.ap().rearrange("(eo p) m -> p eo m", p=P)      # [128, 16, 512]
    wk_r = wk.ap().rearrange("(eo p) m -> p eo m", p=P)
    wv_r = wv.ap().rearrange("(eo p) m -> p eo m", p=P)
    wo_r = wo.ap().rearrange("(n p) e -> p n e", p=P)        # [128, 16, 512]

    with tile.TileContext(nc) as tc:
        with tc.tile_pool(name="const", bufs=1) as const_pool:
            ident = const_pool.tile([P, P], BF16)
            make_identity(nc, ident)
            # multiplicative causal mask: 1 on/below diagonal, 0 above
            dmask = const_pool.tile([P, P], BF16)
            nc.gpsimd.memset(dmask, 1.0)
            nc.gpsimd.affine_select(
                out=dmask, in_=dmask, compare_op=mybir.AluOpType.is_ge,
                fill=0.0, base=0, pattern=[[-1, P]], channel_multiplier=1)
            cos_sb = const_pool.tile([P, S], F32)
            sin_sb = const_pool.tile([P, S], F32)
            cos_srcs = (cosT.ap(), sinT.ap())

            loop_cm = tc.For_i(0, loop_trips, 1) if loop_trips else \
                contextlib.nullcontext()
            with loop_cm:
                _build_body(nc, tc, reps, with_cc, out, ag_in,
                            ag_out1, ag_out2, cos_sb, sin_sb,
                            cos_srcs, xqT_r, xkT_r, wq_r, wk_r, wv_r, wo_r,
                            parts=parts, no_sm=no_sm)

    nc.compile()
    return nc


def _build_body(nc, tc, reps, with_cc, out, ag_in, ag_out1, ag_out2,
                cos_sb, sin_sb, cos_srcs, xqT_r, xkT_r,
                wq_r, wk_r, wv_r, wo_r, parts=("p1", "p2", "p3"),
                no_sm=False):
    for rep in range(reps):
        sfx = f"_r{rep}" if reps > 1 else ""
        with tc.tile_pool(name="qkv" + sfx, bufs=1) as qkv_pool:
            qT_sc = [qkv_pool.tile([P, NH_LOC, 512], BF16, name=f"qT{sc}")
                     for sc in range(4)]
            kT_sc = [qkv_pool.tile([P, NH_LOC, 512], BF16, name=f"kT{sc}")
                     for sc in range(4)]
            v_sc = [qkv_pool.tile([P, 4, NH_LOC * H], BF16, name=f"v{sc}")
                    for sc in range(4)]

            if "p1" in parts:
                _phase1(nc, tc, qT_sc, kT_sc, v_sc, cos_sb, sin_sb, cos_srcs,
                        xqT_r, xkT_r, wq_r, wk_r, wv_r, sfx)

            # prefetch WO for phase 3 (SBUF freed by phase-1 pools)
            with tc.tile_pool(name="wo_pool" + sfx, bufs=1) as wo_pool:
                wo_sb = wo_pool.tile([P, N, ECHUNK], BF16)
                nc.sync.dma_start(wo_sb[:], wo_r)

                # consts for the transposed-score attention
                with tc.tile_pool(name="c2" + sfx, bufs=1) as c2_pool:
                    maskT = c2_pool.tile([P, P], BF16)
                    nc.gpsimd.memset(maskT, 1.0)
                    # keep where s - t >= 0  (upper triangle incl. diagonal)
                    nc.gpsimd.affine_select(
                        out=maskT, in_=maskT,
                        compare_op=mybir.AluOpType.is_ge, fill=0.0,
                        base=0, pattern=[[1, P]], channel_multiplier=-1)
                    ones_col = c2_pool.tile([P, 1], BF16)
                    nc.gpsimd.memset(ones_col, 1.0)
                    ones_row = c2_pool.tile([1, P], F32)
                    nc.gpsimd.memset(ones_row, 1.0)

                    with (
                        tc.tile_pool(name="elt" + sfx,
                                     bufs=POOL_CFG["elt"]) as elt_pool,
                        tc.tile_pool(name="rc" + sfx, bufs=2) as rc_pool,
                        tc.tile_pool(name="bc" + sfx, bufs=2) as bc_pool,
                        tc.tile_pool(name="at" + sfx, bufs=2) as at_pool,
                        tc.tile_pool(name="lt_psum" + sfx,
                                     bufs=POOL_CFG["lt"],
                                     space="PSUM") as lt_psum,
                        tc.tile_pool(name="av_psum" + sfx,
                                     bufs=POOL_CFG["av"],
                                     space="PSUM") as av_psum,
                        tc.tile_pool(name="sm_psum" + sfx,
                                     bufs=POOL_CFG["sm"],
                                     space="PSUM") as sm_psum,
                        tc.tile_pool(name="bc_psum" + sfx,
                                     bufs=POOL_CFG["bc"],
                                     space="PSUM") as bc_psum,
                    ):
                        pools = (elt_pool, rc_pool, bc_pool, at_pool,
                                 lt_psum, av_psum, sm_psum, bc_psum)
                        consts = (maskT, ones_col, ones_row)
                        if "p2" in parts:
                            _attn_heads_v2(nc, tc, pools, [0, 1, 2], qT_sc,
                                           kT_sc, v_sc, consts, ag_in,
                                           no_sm=no_sm)
                        if with_cc:
                            nc.gpsimd.collective_compute(
                                "AllGather", mybir.AluOpType.bypass,
                                replica_groups=REPLICA_GROUPS,
                                ins=[ag_in.ap()[0:3 * H, :].opt()],
                                outs=[ag_out1.ap().opt()])
                        if "p2" in parts:
                            _attn_heads_v2(nc, tc, pools, [3], qT_sc, kT_sc,
                                           v_sc, consts, ag_in, no_sm=no_sm)
                        if with_cc:
                            nc.gpsimd.collective_compute(
                                "AllGather", mybir.AluOpType.bypass,
                                replica_groups=REPLICA_GROUPS,
                                ins=[ag_in.ap()[3 * H:4 * H, :].opt()],
                                outs=[ag_out2.ap().opt()])

                # ------------ phase 3: output projection -------------------
                if "p3" not in parts:
                    continue
                with (
                    tc.tile_pool(name="ao" + sfx, bufs=1) as ao_pool,
                    tc.tile_pool(name="osb" + sfx, bufs=3) as o_pool,
                    tc.tile_pool(name="o_psum" + sfx, bufs=4,
                                 space="PSUM") as op_psum,
                ):
                    # chunked loads: heads from ag_out1 (ready after the
                    # first AllGather) stream in s-chunks first; the four
                    # ag_out2 heads follow on and are accumulated last in
                    # each chain, so the 12 ag1 matmuls overlap AG2.
                    ao_sb = ao_pool.tile([P, N, S], BF16)
                    for c in range(4):
                        csl = slice(c * 512, (c + 1) * 512)
                        for l in range(3):
                            for r in range(4):
                                nc.sync.dma_start(
                                    ao_sb[:, r * NH_LOC + l, csl],
                                    ag_out1[r][l * P:(l + 1) * P, csl])
                    for c in range(4):
                        csl = slice(c * 512, (c + 1) * 512)
                        for r in range(4):
                            nc.sync.dma_start(
                                ao_sb[:, r * NH_LOC + 3, csl],
                                ag_out2[r][0:P, csl])
                    n_ao1 = [r * NH_LOC + l for l in range(3) for r in range(4)]
                    n_ao2 = [r * NH_LOC + 3 for r in range(4)]
                    n_order = n_ao1 + n_ao2
                    for st in range(ST):
                        ssl = slice(st * P, (st + 1) * P)
                        pso = op_psum.tile([P, ECHUNK], F32, tag="pso")
                        for i, n in enumerate(n_order):
                            nc.tensor.matmul(
                                pso, ao_sb[:, n, ssl], wo_sb[:, n, :],
                                start=(i == 0), stop=(i == N - 1))
                        osb = o_pool.tile([P, ECHUNK], F32, tag="osb")
                        nc.scalar.copy(out=osb[:], in_=pso[:])
                        nc.sync.dma_start(out.ap()[ssl, :], osb[:])


_NC_CACHE = None


def _get_nc():
    global _NC_CACHE
    if _NC_CACHE is None:
        _NC_CACHE = _build()
    return _NC_CACHE


def make_in_maps(x_q, x_kv, WQ, WK, WV, WO):
    bf = ml_dtypes.bfloat16
    cosT, sinT = _rope_tables()
    wo_flat = WO.reshape(N * H, E)

    in_maps = []
    xT_cache = {}
    for c in range(NCORES):
        b, hg = c // 4, c % 4
        hsl = slice(hg * NH_LOC, (hg + 1) * NH_LOC)
        esl = slice(hg * ECHUNK, (hg + 1) * ECHUNK)
        if b not in xT_cache:
            xT_cache[b] = (
                np.ascontiguousarray(x_q[b].T.astype(bf)),
                np.ascontiguousarray(x_kv[b].T.astype(bf)),
            )
        xqTb, xkTb = xT_cache[b]
        in_maps.append({
            "xqT": xqTb,
            "xkT": xkTb,
            "wq": np.ascontiguousarray(WQ[:, hsl, :].reshape(E, NH_LOC * H).astype(bf)),
            "wk": np.ascontiguousarray(WK[:, hsl, :].reshape(E, NH_LOC * H).astype(bf)),
            "wv": np.ascontiguousarray(WV[:, hsl, :].reshape(E, NH_LOC * H).astype(bf)),
            "wo": np.ascontiguousarray(wo_flat[:, esl].astype(bf)),
            "cosT": cosT,
            "sinT": sinT,
        })
    return in_maps


def kernel(x_q, x_kv, WQ, WK, WV, WO):
    global LAST_RESULTS
    in_maps = make_in_maps(x_q, x_kv, WQ, WK, WV, WO)
    nc = _get_nc()
    res = run_bass_kernel_spmd(nc, in_maps, core_ids=list(range(NCORES)),
                               trace=TRACE)
    LAST_RESULTS = res

    out = np.empty((B, S, E), dtype=np.float32)
    for c in range(NCORES):
        b, j = c // 4, c % 4
        out[b, :, j * ECHUNK:(j + 1) * ECHUNK] = res.results[c]["out"]
    return out



# revision 26
# speedup vs baseline: 1.0261x; 1.0261x over previous
"""Distributed Trainium2 kernel for causal multi-head attention with RoPE.

Problem: B=2, S=2048, E=2048, N=16 heads, H=128 head-dim.
Sharding: 8 cores = 2 (batch, data axis) x 4 (head groups, model axis).
Each core:
  phase 1: Q/K/V projections for its 4 heads (bf16 matmuls, f32 accum),
           RoPE applied to q^T/k^T in [H, S] layout.
  phase 2: causal attention per head (exact softmax per 128-row block,
           no max subtraction -- logits are O(5) so exp is safe in f32).
           Row-blocks are processed in pairs so the pv stage uses N=256
           matmuls (half the LDWEIGHTS+MATMUL count).
  phase 3: two AllGathers inside each 4-core group (heads 0-1 after they
           finish, heads 2-3 at the end) collect a^T for all 16 heads;
           the first AG overlaps the second half of phase 2. Each core
           then computes the output projection for a different 512-wide
           E-slice (its WO shard) over all s, producing o[2048, 512] f32.
Host side re-assembles the 8 [2048, 512] E-chunks into [2, 2048, 2048].
"""

import contextlib

import numpy as np
import ml_dtypes

import concourse.bass as bass
import concourse.mybir as mybir
import concourse.tile as tile
from concourse import bacc
from concourse.bass_utils import run_bass_kernel_spmd
from concourse.masks import make_identity, make_causal_mask

B, S, E, N, H = 2, 2048, 2048, 16, 128
P = 128
NCORES = 8
NH_LOC = N // 4          # 4 heads per core
ECHUNK = E // 4          # 512 output-embedding columns per core
EC = E // P              # 16 contraction chunks
ST = S // P              # 16 seq tiles of 128
F32 = mybir.dt.float32
BF16 = mybir.dt.bfloat16

REPLICA_GROUPS = [[0, 1, 2, 3], [4, 5, 6, 7]]

TRACE = False
LAST_RESULTS = None
PT_VIA_DMA = False       # transpose P via DMA xbar instead of PE+copy
POOL_CFG = {"elt": 6, "lt": 2, "av": 3, "sm": 2, "bc": 1}
VERSION = 3              # 2 = phased, 3 = per-s-chunk interleaved pipeline


def _rope_tables():
    """cos^T / sin^T tables [H, S] f32, sign-folded and scaled by 128**-0.25.

    Matches reference.sine_table computed in f32 (angles formed with f32
    arithmetic, sin/cos evaluated in f64 of the f32 angle).
    """
    fraction = np.arange(0, H, 2, dtype=np.float32) / np.float32(H)
    timescale = np.float32(10000.0) ** fraction
    inv = (np.float32(1.0) / timescale).astype(np.float32)
    ang = (np.arange(S, dtype=np.float32)[:, None] * inv[None, :]).astype(np.float32)
    ang = np.concatenate([ang, ang], axis=1)        # [S, H]
    sin = np.sin(ang.astype(np.float64))
    cos = np.cos(ang.astype(np.float64))
    scale = float(H) ** -0.25
    cosT = (cos.T * scale).astype(np.float32)        # [H, S]
    sinT = (sin.T * scale).astype(np.float32)
    sinT[:H // 2] *= -1.0                            # sign of rotate_half folded in
    return np.ascontiguousarray(cosT), np.ascontiguousarray(sinT)


def _phase1(nc, tc, qT_sc, kT_sc, v_sc, cos_sb, sin_sb, cos_srcs,
            xqT_r, xkT_r, wq_r, wk_r, wv_r, sfx=""):
    """QKV proj + RoPE -> per-s-chunk qT/kT [h, n, 512] and v [t, tt, nh]."""
    with (
        tc.tile_pool(name="wqkv" + sfx, bufs=1) as w_pool,
        tc.tile_pool(name="xin" + sfx, bufs=2) as x_pool,
        tc.tile_pool(name="rope_tmp" + sfx, bufs=3) as tmp_pool,
        tc.tile_pool(name="ph1_psum" + sfx, bufs=4, space="PSUM") as pp1,
    ):
        # per-4-ec-chunk weight loads: the first q chain starts after a
        # 0.5MB chunk instead of the full 2MB tensor
        wq_sb = w_pool.tile([P, EC, NH_LOC * H], BF16)
        wk_sb = w_pool.tile([P, EC, NH_LOC * H], BF16)
        for j in range(4):
            cs = slice(4 * j, 4 * j + 4)
            nc.sync.dma_start(wq_sb[:, cs, :], wq_r[:, cs, :])
            nc.sync.dma_start(wk_sb[:, cs, :], wk_r[:, cs, :])
        wv_sb = w_pool.tile([P, EC, NH_LOC * H], BF16)

        def rope(ps, dst, cols):
            """dst = ps*cos + shift128(ps)*sin  (bf16 out), cols into S."""
            w = cols.stop - cols.start
            t_sin = tmp_pool.tile([P, 512], F32, tag="t_sin")
            nc.vector.tensor_tensor(
                out=t_sin[0:64, :w], in0=ps[64:128, :w],
                in1=sin_sb[0:64, cols], op=mybir.AluOpType.mult)
            nc.vector.tensor_tensor(
                out=t_sin[64:128, :w], in0=ps[0:64, :w],
                in1=sin_sb[64:128, cols], op=mybir.AluOpType.mult)
            t_cos = tmp_pool.tile([P, 512], F32, tag="t_cos")
            nc.vector.tensor_tensor(
                out=t_cos[:, :w], in0=ps[:, :w],
                in1=cos_sb[:, cols], op=mybir.AluOpType.mult)
            nc.vector.tensor_add(out=dst, in0=t_cos[:, :w], in1=t_sin[:, :w])

        for sc in range(4):                     # 512-col s-chunks
            cols = slice(sc * 512, (sc + 1) * 512)
            # 4 strip tiles of 4 e-chunks each -> matmuls start after the
            # first strip lands instead of the full 2MB transfer.
            xq_st, xk_st = [], []
            for j in range(4):
                xq = x_pool.tile([P, 4, 512], BF16, tag=f"xq{j}")
                nc.sync.dma_start(xq[:], xqT_r[:, 4 * j:4 * j + 4, cols])
                xq_st.append(xq)
                xk = x_pool.tile([P, 4, 512], BF16, tag=f"xk{j}")
                nc.sync.dma_start(xk[:], xkT_r[:, 4 * j:4 * j + 4, cols])
                xk_st.append(xk)
            if sc == 0:
                # deferred loads: not needed until RoPE / the v chains,
                # so they queue behind the first q/k strips
                nc.sync.dma_start(cos_sb[:], cos_srcs[0])
                nc.sync.dma_start(sin_sb[:], cos_srcs[1])
                nc.sync.dma_start(wv_sb[:], wv_r)

            for n in range(NH_LOC):
                hs = slice(n * H, (n + 1) * H)
                psq = pp1.tile([P, 512], F32, tag="ps")
                for ec in range(EC):
                    nc.tensor.matmul(
                        psq, wq_sb[:, ec, hs], xq_st[ec // 4][:, ec % 4, :],
                        start=(ec == 0), stop=(ec == EC - 1))
                rope(psq, qT_sc[sc][:, n, :], cols)
                psk = pp1.tile([P, 512], F32, tag="ps")
                for ec in range(EC):
                    nc.tensor.matmul(
                        psk, wk_sb[:, ec, hs], xk_st[ec // 4][:, ec % 4, :],
                        start=(ec == 0), stop=(ec == EC - 1))
                rope(psk, kT_sc[sc][:, n, :], cols)

            for tt in range(4):                 # v for 4 t-tiles
                tsl = slice(tt * P, (tt + 1) * P)
                psv = pp1.tile([P, 512], F32, tag="ps")
                for ec in range(EC):
                    nc.tensor.matmul(
                        psv, xk_st[ec // 4][:, ec % 4, tsl], wv_sb[:, ec, :],
                        start=(ec == 0), stop=(ec == EC - 1))
                nc.scalar.copy(out=v_sc[sc][:, tt, :], in_=psv[:])


def _attn_heads_v2(nc, tc, pools, heads, qT_sc, kT_sc, v_sc, consts, ag_in,
                   no_sm=False):
    """Causal attention, transposed-score formulation.

    Scores are computed directly as L^T[t, s] (lhsT = k^T block, rhs = q^T
    chunk, N=512), exp goes PSUM -> SBUF bf16 with no transpose/copy, and
    a^T[h, s] accumulates with wide pv matmuls (lhsT = v block).  Row-sums
    (softmax denominators) come from ones-column matmuls accumulated next
    to the pv chain; normalization multiplies a^T by an outer-product
    broadcast (K=1 matmul) of the reciprocals.
    """
    (elt_pool, rc_pool, bc_pool, at_pool,
     lt_psum, av_psum, sm_psum, bc_psum) = pools
    maskT, ones_col, ones_row = consts

    def flush(pend):
        """Deferred normalization: by now rc is long since computed on DVE,
        so the bc broadcast matmul never stalls the PE."""
        n, sc, avp, rc = pend
        bcp = bc_psum.tile([P, 512], F32, tag="bc")
        nc.tensor.matmul(bcp, ones_row[:], rc[:], start=True, stop=True)
        bcs = bc_pool.tile([P, 512], F32, tag="bcs")
        nc.vector.tensor_copy(out=bcs[:], in_=bcp[:])
        at = at_pool.tile([P, 512], BF16, tag="at")
        nc.vector.tensor_mul(out=at[:], in0=avp[:], in1=bcs[:])
        nc.sync.dma_start(
            ag_in.ap()[n * P:(n + 1) * P, sc * 512:(sc + 1) * 512], at[:])

    pending = None
    for sc in range(4):                         # 512-wide s-chunks
        for n in heads:
            hs = slice(n * H, (n + 1) * H)
            ntb = 4 * sc + 4                    # causal t-blocks for chunk
            avp = av_psum.tile([P, 512], F32, tag="av")
            smp = sm_psum.tile([1, 512], F32, tag="sm")
            for tb in range(ntb):
                w = 512 if tb < 4 * sc else 512 - (tb - 4 * sc) * P
                col0 = 512 - w
                kblk = kT_sc[tb // 4][:, n, (tb % 4) * P:(tb % 4 + 1) * P]
                lt = lt_psum.tile([P, 512], F32, tag="lt")
                nc.tensor.matmul(
                    lt[:, :w], kblk, qT_sc[sc][:, n, col0:512],
                    start=True, stop=True)
                elt = elt_pool.tile([P, 512], BF16, tag="elt")
                nc.scalar.activation(
                    out=elt[:, :w], in_=lt[:, :w],
                    func=mybir.ActivationFunctionType.Exp)
                if tb >= 4 * sc:                # diagonal: zero t > s
                    nc.vector.tensor_mul(
                        out=elt[:, 0:P], in0=elt[:, 0:P], in1=maskT[:])
                nc.tensor.matmul(
                    avp[:, col0:512], v_sc[tb // 4][:, tb % 4, hs],
                    elt[:, :w],
                    start=(tb == 0), stop=(tb == ntb - 1),
                    skip_group_check=True)
                if not no_sm:
                    nc.tensor.matmul(
                        smp[:, col0:512], ones_col[:], elt[:, :w],
                        start=(tb == 0), stop=(tb == ntb - 1),
                        skip_group_check=True)
            if no_sm:
                at = at_pool.tile([P, 512], BF16, tag="at")
                nc.vector.tensor_copy(out=at[:], in_=avp[:])
                nc.sync.dma_start(
                    ag_in.ap()[n * P:(n + 1) * P,
                               sc * 512:(sc + 1) * 512], at[:])
                continue
            rc = rc_pool.tile([1, 512], F32, tag="rc")
            nc.vector.reciprocal(rc[:], smp[:])
            if pending is not None:
                flush(pending)
            pending = (n, sc, avp, rc)
    if pending is not None:
        flush(pending)


def _attn_heads(nc, tc, pools, heads, qT_sc, kT_sc, v_sc, dmask, ident, ag_in):
    """Causal attention for the given local heads; writes a^T into ag_in."""
    (probs_pool, ssum_pool, ptsb_pool, atsb_pool,
     sc_psum, pt_psum, av_psum) = pools

    def qT(n, rows):
        return qT_sc[rows.start // 512][:, n,
                                        rows.start % 512:
                                        rows.start % 512 + (rows.stop - rows.start)]

    def softmax_block(n, i, probs):
        """logits + exp + causal 0/1-mask + normalize for row-block i.

        The diagonal block is masked multiplicatively on the cheap bf16
        side (exp of unmasked logits is finite, ~e^6 max), and its chunk
        row-sum is recomputed from the masked bf16 probs.
        """
        span = (i + 1) * P
        nch = (span + 511) // 512
        dj = (span - P) // 512                  # chunk holding the diagonal
        sums = ssum_pool.tile([P, 4], F32, tag="sums")
        rows = slice(i * P, (i + 1) * P)
        for j in range(nch):
            w = min(512, span - j * 512)
            sp = sc_psum.tile([P, 512], F32, tag="sp")
            nc.tensor.matmul(
                sp[:, :w], qT(n, rows), kT_sc[j][:, n, :w],
                start=True, stop=True)
            nc.scalar.activation(
                out=probs[:, j * 512:j * 512 + w],
                in_=sp[:, :w],
                func=mybir.ActivationFunctionType.Exp,
                accum_out=(None if j == dj else sums[:, j:j + 1]))
        off = (i % 4) * P
        dsl = slice(span - P, span)
        nc.vector.tensor_mul(
            out=probs[:, dsl], in0=probs[:, dsl], in1=dmask[:])
        wd = min(512, span - dj * 512)
        nc.vector.reduce_sum(
            out=sums[:, dj:dj + 1], in_=probs[:, dj * 512:dj * 512 + wd],
            axis=mybir.AxisListType.X)
        recip = ssum_pool.tile([P, 1], F32, tag="recip")
        if nch > 1:
            tot = ssum_pool.tile([P, 1], F32, tag="tot")
            nc.vector.reduce_sum(
                out=tot[:], in_=sums[:, :nch], axis=mybir.AxisListType.X)
            nc.vector.reciprocal(recip[:], tot[:])
        else:
            nc.vector.reciprocal(recip[:], sums[:, 0:1])
        for j in range(nch):
            w = min(512, span - j * 512)
            nc.vector.tensor_scalar_mul(
                probs[:, j * 512:j * 512 + w],
                probs[:, j * 512:j * 512 + w], recip[:])

    for m in range(ST // 2):                    # paired row-blocks (2m, 2m+1)
        for n in heads:
            hs = slice(n * H, (n + 1) * H)
            i0, i1 = 2 * m, 2 * m + 1
            probs0 = probs_pool.tile([P, S], BF16, tag="probs0")
            probs1 = probs_pool.tile([P, S], BF16, tag="probs1")
            softmax_block(n, i0, probs0)
            softmax_block(n, i1, probs1)

            # a^T = v.T @ P^T for both blocks: pv rhs is [t, 256]
            # (128 cols per block); i1's diagonal t-block rides in the
            # last transpose group (nblk0 is odd, so it has a free slot).
            avp = av_psum.tile([P, 2 * P], F32, tag="av")
            nblk0 = i0 + 1
            for g in range(0, nblk0, 2):        # 2 t-blocks per transpose grp
                gw = min(2, nblk0 - g)
                last = (g + 2 >= nblk0)
                pts = ptsb_pool.tile([P, 512], BF16, tag="pts")
                if PT_VIA_DMA:
                    for q in range(gw):
                        tb = g + q
                        nc.sync.dma_start(
                            out=pts[:, q * 256:q * 256 + P],
                            in_=probs0[:, tb * P:(tb + 1) * P], transpose=True)
                        nc.sync.dma_start(
                            out=pts[:, q * 256 + P:q * 256 + 2 * P],
                            in_=probs1[:, tb * P:(tb + 1) * P], transpose=True)
                    if last:
                        nc.sync.dma_start(
                            out=pts[:, gw * 256:gw * 256 + P],
                            in_=probs1[:, i1 * P:(i1 + 1) * P], transpose=True)
                else:
                    ptp = pt_psum.tile([P, 512], BF16, tag="pt")
                    for q in range(gw):
                        tb = g + q
                        nc.tensor.transpose(
                            ptp[:, q * 256:q * 256 + P],
                            probs0[:, tb * P:(tb + 1) * P], ident)
                        nc.tensor.transpose(
                            ptp[:, q * 256 + P:q * 256 + 2 * P],
                            probs1[:, tb * P:(tb + 1) * P], ident)
                    cw = gw * 256
                    if last:                    # diag t-block of i1
                        nc.tensor.transpose(
                            ptp[:, cw:cw + P],
                            probs1[:, i1 * P:(i1 + 1) * P], ident)
                        cw += P
                    nc.any.tensor_copy(out=pts[:, :cw], in_=ptp[:, :cw])
                for q in range(gw):
                    tb = g + q
                    nc.tensor.matmul(
                        avp, v_sc[tb // 4][:, tb % 4, hs],
                        pts[:, q * 256:(q + 1) * 256],
                        start=(tb == 0), stop=False, skip_group_check=True)
                if last:
                    nc.tensor.matmul(
                        avp[:, P:2 * P], v_sc[i1 // 4][:, i1 % 4, hs],
                        pts[:, gw * 256:gw * 256 + P],
                        start=False, stop=True, skip_group_check=True)

            at = atsb_pool.tile([P, 2 * P], BF16, tag="at")
            nc.any.tensor_copy(out=at[:], in_=avp[:])
            nc.sync.dma_start(
                ag_in.ap()[n * P:(n + 1) * P, i0 * P:(i0 + 2) * P], at[:])


def _build_body_v3(nc, tc, with_cc, out, ag_in, ag_out_sc, cos_srcs,
                   xqT_r, xkT_r, wq_r, wk_r, wv_r, wo_r,
                   parts=("p1", "p2", "p3"), no_sm=False, sfx=""):
    """Per-s-chunk interleaved pipeline.

    For each 512-wide s-chunk: QKV projection+RoPE, then causal attention
    for the chunk (possible because chunk sc only attends to t <= sc), an
    AllGather of just this chunk's a^T, and the output projection of the
    PREVIOUS chunk (giving its AllGather a full chunk of compute to hide
    behind).  Attention's exp/normalization (Act+DVE) ride under the next
    chunk's projection matmuls, and the PE never idles on collectives.
    """
    with (
        tc.tile_pool(name="c3" + sfx, bufs=1) as c_pool,
        tc.tile_pool(name="w3" + sfx, bufs=1) as w_pool,
        tc.tile_pool(name="qkv3" + sfx, bufs=1) as qkv_pool,
        tc.tile_pool(name="x3" + sfx, bufs=1) as x_pool,
        tc.tile_pool(name="tmp3" + sfx, bufs=2) as tmp_pool,
        tc.tile_pool(name="elt3" + sfx, bufs=2) as elt_pool,
        tc.tile_pool(name="rc3" + sfx, bufs=2) as rc_pool,
        tc.tile_pool(name="bc3" + sfx, bufs=2) as bc_pool,
        tc.tile_pool(name="at3" + sfx, bufs=2) as at_pool,
        tc.tile_pool(name="ao3" + sfx, bufs=1) as ao_pool,
        tc.tile_pool(name="o3" + sfx, bufs=2) as o_pool,
        tc.tile_pool(name="ps3" + sfx, bufs=4, space="PSUM") as ps_pool,
        tc.tile_pool(name="av3" + sfx, bufs=3, space="PSUM") as av_psum,
        tc.tile_pool(name="sm3" + sfx, bufs=1, space="PSUM") as sm_psum,
    ):
        # ---- constants ----
        maskT = c_pool.tile([P, P], BF16)
        nc.gpsimd.memset(maskT, 1.0)
        nc.gpsimd.affine_select(
            out=maskT, in_=maskT, compare_op=mybir.AluOpType.is_ge,
            fill=0.0, base=0, pattern=[[1, P]], channel_multiplier=-1)
        ones_col = c_pool.tile([P, 1], BF16)
        nc.gpsimd.memset(ones_col, 1.0)
        ones_row = c_pool.tile([1, P], BF16)
        nc.gpsimd.memset(ones_row, 1.0)
        cos_sb = c_pool.tile([P, S], F32)
        sin_sb = c_pool.tile([P, S], F32)

        # ---- weights (chunked so the first chain starts early) ----
        wq_sb = w_pool.tile([P, EC, NH_LOC * H], BF16)
        wk_sb = w_pool.tile([P, EC, NH_LOC * H], BF16)
        wv_sb = w_pool.tile([P, EC, NH_LOC * H], BF16)
        wo_sb = w_pool.tile([P, N, ECHUNK], BF16)
        for j in range(4):
            cs = slice(4 * j, 4 * j + 4)
            nc.sync.dma_start(wq_sb[:, cs, :], wq_r[:, cs, :])
            nc.sync.dma_start(wk_sb[:, cs, :], wk_r[:, cs, :])

        kT_sc = [qkv_pool.tile([P, NH_LOC, 512], BF16, name=f"kT3{c}")
                 for c in range(4)]
        v_sc = [qkv_pool.tile([P, 4, NH_LOC * H], BF16, name=f"v3{c}")
                for c in range(4)]

        def rope(ps, dst, cols):
            w = cols.stop - cols.start
            t_sin = tmp_pool.tile([P, 512], F32, tag="t_sin")
            nc.vector.tensor_tensor(
                out=t_sin[0:64, :w], in0=ps[64:128, :w],
                in1=sin_sb[0:64, cols], op=mybir.AluOpType.mult)
            nc.vector.tensor_tensor(
                out=t_sin[64:128, :w], in0=ps[0:64, :w],
                in1=sin_sb[64:128, cols], op=mybir.AluOpType.mult)
            t_cos = tmp_pool.tile([P, 512], F32, tag="t_cos")
            nc.vector.tensor_tensor(
                out=t_cos[:, :w], in0=ps[:, :w],
                in1=cos_sb[:, cols], op=mybir.AluOpType.mult)
            nc.vector.tensor_add(out=dst, in0=t_cos[:, :w], in1=t_sin[:, :w])

        def flush(pend):
            n, c, avp, rc = pend
            bcp = ps_pool.tile([P, 512], F32, tag="ps")
            nc.tensor.matmul(bcp, ones_row[:], rc[:], start=True, stop=True)
            bcs = bc_pool.tile([P, 512], F32, tag="bcs")
            nc.vector.tensor_copy(out=bcs[:], in_=bcp[:])
            at = at_pool.tile([P, 512], BF16, tag="at")
            nc.vector.tensor_mul(out=at[:], in0=avp[:], in1=bcs[:])
            nc.sync.dma_start(
                ag_in.ap()[c, n * P:(n + 1) * P, :], at[:])

        def phase3_chunk(c):
            ao = ao_pool.tile([P, N, 512], BF16, tag="ao")
            for l in range(NH_LOC):
                for r in range(4):
                    nc.sync.dma_start(
                        ao[:, r * NH_LOC + l, :],
                        ag_out_sc[c].ap()[r, l * P:(l + 1) * P, :])
            for stl in range(4):
                pso = ps_pool.tile([P, ECHUNK], F32, tag="ps")
                for n in range(N):
                    nc.tensor.matmul(
                        pso, ao[:, n, stl * P:(stl + 1) * P], wo_sb[:, n, :],
                        start=(n == 0), stop=(n == N - 1))
                osb = o_pool.tile([P, ECHUNK], F32, tag="osb")
                nc.scalar.copy(out=osb[:], in_=pso[:])
                st = 4 * c + stl
                nc.sync.dma_start(out.ap()[st * P:(st + 1) * P, :], osb[:])

        for sc in range(4):
            cols = slice(sc * 512, (sc + 1) * 512)
            # ---------------- phase 1 chunk ----------------
            if "p1" in parts:
                qT = qkv_pool.tile([P, NH_LOC, 512], BF16, tag="qT", bufs=2)
                xq_st, xk_st = [], []
                for j in range(4):
                    xq = x_pool.tile([P, 4, 512], BF16, tag=f"xq{j}")
                    nc.sync.dma_start(xq[:], xqT_r[:, 4 * j:4 * j + 4, cols])
                    xq_st.append(xq)
                    xk = x_pool.tile([P, 4, 512], BF16, tag=f"xk{j}")
                    nc.sync.dma_start(xk[:], xkT_r[:, 4 * j:4 * j + 4, cols])
                    xk_st.append(xk)
                if sc == 0:
                    nc.sync.dma_start(cos_sb[:], cos_srcs[0])
                    nc.sync.dma_start(sin_sb[:], cos_srcs[1])
                    nc.sync.dma_start(wv_sb[:], wv_r)
                if sc == 1:
                    nc.sync.dma_start(wo_sb[:], wo_r)

                for n in range(NH_LOC):
                    hs = slice(n * H, (n + 1) * H)
                    psq = ps_pool.tile([P, 512], F32, tag="ps")
                    for ec in range(EC):
                        nc.tensor.matmul(
                            psq, wq_sb[:, ec, hs],
                            xq_st[ec // 4][:, ec % 4, :],
                            start=(ec == 0), stop=(ec == EC - 1))
                    rope(psq, qT[:, n, :], cols)
                    psk = ps_pool.tile([P, 512], F32, tag="ps")
                    for ec in range(EC):
                        nc.tensor.matmul(
                            psk, wk_sb[:, ec, hs],
                            xk_st[ec // 4][:, ec % 4, :],
                            start=(ec == 0), stop=(ec == EC - 1))
                    rope(psk, kT_sc[sc][:, n, :], cols)
                for tt in range(4):
                    tsl = slice(tt * P, (tt + 1) * P)
                    psv = ps_pool.tile([P, 512], F32, tag="ps")
                    for ec in range(EC):
                        nc.tensor.matmul(
                            psv, xk_st[ec // 4][:, ec % 4, tsl],
                            wv_sb[:, ec, :],
                            start=(ec == 0), stop=(ec == EC - 1))
                    nc.scalar.copy(out=v_sc[sc][:, tt, :], in_=psv[:])

            # ---------------- attention chunk ----------------
            # Three clean passes per head (scores+exp, pv, softmax-sum) so
            # the PE runs long streams without cycling three stationary
            # operands per t-block (weight-buffer thrash on HW).
            if "p2" in parts:
                pending = None
                for n in range(NH_LOC):
                    hs = slice(n * H, (n + 1) * H)
                    ntb = 4 * sc + 4
                    eltr = [elt_pool.tile([P, 512], BF16, tag=f"elt{tb}",
                                          bufs=1, name=f"elt{tb}")
                            for tb in range(ntb)]

                    def wcol(tb):
                        w = 512 if tb < 4 * sc else 512 - (tb - 4 * sc) * P
                        return w, 512 - w

                    for tb in range(ntb):       # pass 1: scores + exp
                        w, col0 = wcol(tb)
                        kblk = kT_sc[tb // 4][:, n,
                                              (tb % 4) * P:(tb % 4 + 1) * P]
                        lt = ps_pool.tile([P, 512], F32, tag="ps")
                        nc.tensor.matmul(
                            lt[:, :w], kblk, qT[:, n, col0:512],
                            start=True, stop=True)
                        nc.scalar.activation(
                            out=eltr[tb][:, :w], in_=lt[:, :w],
                            func=mybir.ActivationFunctionType.Exp)
                        if tb >= 4 * sc:
                            nc.vector.tensor_mul(
                                out=eltr[tb][:, 0:P], in0=eltr[tb][:, 0:P],
                                in1=maskT[:])
                    avp = av_psum.tile([P, 512], F32, tag="av")
                    for tb in range(ntb):       # pass 2: pv
                        w, col0 = wcol(tb)
                        nc.tensor.matmul(
                            avp[:, col0:512], v_sc[tb // 4][:, tb % 4, hs],
                            eltr[tb][:, :w],
                            start=(tb == 0), stop=(tb == ntb - 1),
                            skip_group_check=True)
                    if no_sm:
                        at = at_pool.tile([P, 512], BF16, tag="at")
                        nc.vector.tensor_copy(out=at[:], in_=avp[:])
                        nc.sync.dma_start(
                            ag_in.ap()[sc, n * P:(n + 1) * P, :], at[:])
                        continue
                    smp = sm_psum.tile([1, 512], F32, tag="sm")
                    for tb in range(ntb):       # pass 3: denominators
                        w, col0 = wcol(tb)
                        nc.tensor.matmul(
                            smp[:, col0:512], ones_col[:], eltr[tb][:, :w],
                            start=(tb == 0), stop=(tb == ntb - 1),
                            skip_group_check=True)
                    rc = rc_pool.tile([1, 512], BF16, tag="rc")
                    nc.vector.reciprocal(rc[:], smp[:])
                    if pending is not None:
                        flush(pending)
                    pending = (n, sc, avp, rc)
                if pending is not None:
                    flush(pending)

            if with_cc:
                nc.gpsimd.collective_compute(
                    "AllGather", mybir.AluOpType.bypass,
                    replica_groups=REPLICA_GROUPS,
                    ins=[ag_in.ap()[sc].opt()],
                    outs=[ag_out_sc[sc].ap().opt()])

            if "p3" in parts and sc >= 1:
                phase3_chunk(sc - 1)
        if "p3" in parts:
            phase3_chunk(3)


def _build(reps=1, with_cc=True, loop_trips=0, parts=("p1", "p2", "p3"),
           no_sm=False, version=None):
    if version is None:
        version = VERSION
    nc = bacc.Bacc("TRN2", target_bir_lowering=False, debug=False,
                   num_devices=NCORES)

    xqT = nc.dram_tensor("xqT", [E, S], BF16, kind="ExternalInput")
    xkT = nc.dram_tensor("xkT", [E, S], BF16, kind="ExternalInput")
    wq = nc.dram_tensor("wq", [E, NH_LOC * H], BF16, kind="ExternalInput")
    wk = nc.dram_tensor("wk", [E, NH_LOC * H], BF16, kind="ExternalInput")
    wv = nc.dram_tensor("wv", [E, NH_LOC * H], BF16, kind="ExternalInput")
    wo = nc.dram_tensor("wo", [N * H, ECHUNK], BF16, kind="ExternalInput")
    cosT = nc.dram_tensor("cosT", [H, S], F32, kind="ExternalInput")
    sinT = nc.dram_tensor("sinT", [H, S], F32, kind="ExternalInput")
    out = nc.dram_tensor("out", [S, ECHUNK], F32, kind="ExternalOutput")

    xqT_r = xqT.ap().rearrange("(eo p) s -> p eo s", p=P)    # [128, 16, 2048]
    xkT_r = xkT.ap().rearrange("(eo p) s -> p eo s", p=P)
    wq_r = wq.ap().rearrange("(eo p) m -> p eo m", p=P)      # [128, 16, 512]
    wk_r = wk.ap().rearrange("(eo p) m -> p eo m", p=P)
    wv_r = wv.ap().rearrange("(eo p) m -> p eo m", p=P)
    wo_r = wo.ap().rearrange("(n p) e -> p n e", p=P)        # [128, 16, 512]

    if version == 3:
        ag_in = nc.dram_tensor("ag_in", [4, NH_LOC * H, 512], BF16)
        ag_out_sc = [nc.dram_tensor(f"ag_out{c}", [4, NH_LOC * H, 512], BF16)
                     for c in range(4)]
        with tile.TileContext(nc) as tc:
            loop_cm = tc.For_i(0, loop_trips, 1) if loop_trips else \
                contextlib.nullcontext()
            with loop_cm:
                for rep in range(reps):
                    _build_body_v3(nc, tc, with_cc, out, ag_in, ag_out_sc,
                                   (cosT.ap(), sinT.ap()), xqT_r, xkT_r,
                                   wq_r, wk_r, wv_r, wo_r,
                                   parts=parts, no_sm=no_sm,
                                   sfx=(f"_r{rep}" if reps > 1 else ""))
        nc.compile()
        return nc

    ag_in = nc.dram_tensor("ag_in", [NH_LOC * H, S], BF16)
    ag_out1 = nc.dram_tensor("ag_out1", [4, 3 * H, S], BF16)
    ag_out2 = nc.dram_tensor("ag_out2", [4, 1 * H, S], BF16)

    with tile.TileContext(nc) as tc:
        with tc.tile_pool(name="const", bufs=1) as const_pool:
            ident = const_pool.tile([P, P], BF16)
            make_identity(nc, ident)
            # multiplicative causal mask: 1 on/below diagonal, 0 above
            dmask = const_pool.tile([P, P], BF16)
            nc.gpsimd.memset(dmask, 1.0)
            nc.gpsimd.affine_select(
                out=dmask, in_=dmask, compare_op=mybir.AluOpType.is_ge,
                fill=0.0, base=0, pattern=[[-1, P]], channel_multiplier=1)
            cos_sb = const_pool.tile([P, S], F32)
            sin_sb = const_pool.tile([P, S], F32)
            cos_srcs = (cosT.ap(), sinT.ap())

            loop_cm = tc.For_i(0, loop_trips, 1) if loop_trips else \
                contextlib.nullcontext()
            with loop_cm:
                _build_body(nc, tc, reps, with_cc, out, ag_in,
                            ag_out1, ag_out2, cos_sb, sin_sb,
                            cos_srcs, xqT_r, xkT_r, wq_r, wk_r, wv_r, wo_r,
                            parts=parts, no_sm=no_sm)

    nc.compile()
    return nc


def _build_body(nc, tc, reps, with_cc, out, ag_in, ag_out1, ag_out2,
                cos_sb, sin_sb, cos_srcs, xqT_r, xkT_r,
                wq_r, wk_r, wv_r, wo_r, parts=("p1", "p2", "p3"),
                no_sm=False):
    for rep in range(reps):
        sfx = f"_r{rep}" if reps > 1 else ""
        with tc.tile_pool(name="qkv" + sfx, bufs=1) as qkv_pool:
            qT_sc = [qkv_pool.tile([P, NH_LOC, 512], BF16, name=f"qT{sc}")
                     for sc in range(4)]
            kT_sc = [qkv_pool.tile([P, NH_LOC, 512], BF16, name=f"kT{sc}")
                     for sc in range(4)]
            v_sc = [qkv_pool.tile([P, 4, NH_LOC * H], BF16, name=f"v{sc}")
                    for sc in range(4)]

            if "p1" in parts:
                _phase1(nc, tc, qT_sc, kT_sc, v_sc, cos_sb, sin_sb, cos_srcs,
                        xqT_r, xkT_r, wq_r, wk_r, wv_r, sfx)

            # prefetch WO for phase 3 (SBUF freed by phase-1 pools)
            with tc.tile_pool(name="wo_pool" + sfx, bufs=1) as wo_pool:
                wo_sb = wo_pool.tile([P, N, ECHUNK], BF16)
                nc.sync.dma_start(wo_sb[:], wo_r)

                # consts for the transposed-score attention
                with tc.tile_pool(name="c2" + sfx, bufs=1) as c2_pool:
                    maskT = c2_pool.tile([P, P], BF16)
                    nc.gpsimd.memset(maskT, 1.0)
                    # keep where s - t >= 0  (upper triangle incl. diagonal)
                    nc.gpsimd.affine_select(
                        out=maskT, in_=maskT,
                        compare_op=mybir.AluOpType.is_ge, fill=0.0,
                        base=0, pattern=[[1, P]], channel_multiplier=-1)
                    ones_col = c2_pool.tile([P, 1], BF16)
                    nc.gpsimd.memset(ones_col, 1.0)
                    ones_row = c2_pool.tile([1, P], F32)
                    nc.gpsimd.memset(ones_row, 1.0)

                    with (
                        tc.tile_pool(name="elt" + sfx,
                                     bufs=POOL_CFG["elt"]) as elt_pool,
                        tc.tile_pool(name="rc" + sfx, bufs=2) as rc_pool,
                        tc.tile_pool(name="bc" + sfx, bufs=2) as bc_pool,
                        tc.tile_pool(name="at" + sfx, bufs=2) as at_pool,
                        tc.tile_pool(name="lt_psum" + sfx,
                                     bufs=POOL_CFG["lt"],
                                     space="PSUM") as lt_psum,
                        tc.tile_pool(name="av_psum" + sfx,
                                     bufs=POOL_CFG["av"],
                                     space="PSUM") as av_psum,
                        tc.tile_pool(name="sm_psum" + sfx,
                                     bufs=POOL_CFG["sm"],
                                     space="PSUM") as sm_psum,
                        tc.tile_pool(name="bc_psum" + sfx,
                                     bufs=POOL_CFG["bc"],
                                     space="PSUM") as bc_psum,
                    ):
                        pools = (elt_pool, rc_pool, bc_pool, at_pool,
                                 lt_psum, av_psum, sm_psum, bc_psum)
                        consts = (maskT, ones_col, ones_row)
                        if "p2" in parts:
                            _attn_heads_v2(nc, tc, pools, [0, 1, 2], qT_sc,
                                           kT_sc, v_sc, consts, ag_in,
                                           no_sm=no_sm)
                        if with_cc:
                            nc.gpsimd.collective_compute(
                                "AllGather", mybir.AluOpType.bypass,
                                replica_groups=REPLICA_GROUPS,
                                ins=[ag_in.ap()[0:3 * H, :].opt()],
                                outs=[ag_out1.ap().opt()])
                        if "p2" in parts:
                            _attn_heads_v2(nc, tc, pools, [3], qT_sc, kT_sc,
                                           v_sc, consts, ag_in, no_sm=no_sm)
                        if with_cc:
                            nc.gpsimd.collective_compute(
                                "AllGather", mybir.AluOpType.bypass,
                                replica_groups=REPLICA_GROUPS,
                                ins=[ag_in.ap()[3 * H:4 * H, :].opt()],
                                outs=[ag_out2.ap().opt()])

                # ------------ phase 3: output projection -------------------
                if "p3" not in parts:
                    continue
                with (
                    tc.tile_pool(name="ao" + sfx, bufs=1) as ao_pool,
                    tc.tile_pool(name="osb" + sfx, bufs=3) as o_pool,
                    tc.tile_pool(name="o_psum" + sfx, bufs=4,
                                 space="PSUM") as op_psum,
                ):
                    # chunked loads: heads from ag_out1 (ready after the
                    # first AllGather) stream in s-chunks first; the four
                    # ag_out2 heads follow on and are accumulated last in
                    # each chain, so the 12 ag1 matmuls overlap AG2.
                    ao_sb = ao_pool.tile([P, N, S], BF16)
                    for c in range(4):
                        csl = slice(c * 512, (c + 1) * 512)
                        for l in range(3):
                            for r in range(4):
                                nc.sync.dma_start(
                                    ao_sb[:, r * NH_LOC + l, csl],
                                    ag_out1[r][l * P:(l + 1) * P, csl])
                    for c in range(4):
                        csl = slice(c * 512, (c + 1) * 512)
                        for r in range(4):
                            nc.sync.dma_start(
                                ao_sb[:, r * NH_LOC + 3, csl],
                                ag_out2[r][0:P, csl])
                    n_ao1 = [r * NH_LOC + l for l in range(3) for r in range(4)]
                    n_ao2 = [r * NH_LOC + 3 for r in range(4)]
                    n_order = n_ao1 + n_ao2
                    for st in range(ST):
                        ssl = slice(st * P, (st + 1) * P)
                        pso = op_psum.tile([P, ECHUNK], F32, tag="pso")
                        for i, n in enumerate(n_order):
                            nc.tensor.matmul(
                                pso, ao_sb[:, n, ssl], wo_sb[:, n, :],
                                start=(i == 0), stop=(i == N - 1))
                        osb = o_pool.tile([P, ECHUNK], F32, tag="osb")
                        nc.scalar.copy(out=osb[:], in_=pso[:])
                        nc.sync.dma_start(out.ap()[ssl, :], osb[:])


_NC_CACHE = None


def _get_nc():
    global _NC_CACHE
    if _NC_CACHE is None:
        _NC_CACHE = _build()
    return _NC_CACHE


def make_in_maps(x_q, x_kv, WQ, WK, WV, WO):
    bf = ml_dtypes.bfloat16
    cosT, sinT = _rope_tables()
    wo_flat = WO.reshape(N * H, E)

    in_maps = []
    xT_cache = {}
    for c in range(NCORES):
        b, hg = c // 4, c % 4
        hsl = slice(hg * NH_LOC, (hg + 1) * NH_LOC)
        esl = slice(hg * ECHUNK, (hg + 1) * ECHUNK)
        if b not in xT_cache:
            xT_cache[b] = (
                np.ascontiguousarray(x_q[b].T.astype(bf)),
                np.ascontiguousarray(x_kv[b].T.astype(bf)),
            )
        xqTb, xkTb = xT_cache[b]
        in_maps.append({
            "xqT": xqTb,
            "xkT": xkTb,
            "wq": np.ascontiguousarray(WQ[:, hsl, :].reshape(E, NH_LOC * H).astype(bf)),
            "wk": np.ascontiguousarray(WK[:, hsl, :].reshape(E, NH_LOC * H).astype(bf)),
            "wv": np.ascontiguousarray(WV[:, hsl, :].reshape(E, NH_LOC * H).astype(bf)),
            "wo": np.ascontiguousarray(wo_flat[:, esl].astype(bf)),
            "cosT": cosT,
            "sinT": sinT,
        })
    return in_maps


def kernel(x_q, x_kv, WQ, WK, WV, WO):
    global LAST_RESULTS
    in_maps = make_in_maps(x_q, x_kv, WQ, WK, WV, WO)
    nc = _get_nc()
    res = run_bass_kernel_spmd(nc, in_maps, core_ids=list(range(NCORES)),
                               trace=TRACE)
    LAST_RESULTS = res

    out = np.empty((B, S, E), dtype=np.float32)
    for c in range(NCORES):
        b, j = c // 4, c % 4
        out[b, :, j * ECHUNK:(j + 1) * ECHUNK] = res.results[c]["out"]
    return out

